# revision 45
# baseline (speedup 1.0000x reference)
"""AttnBlock (GroupNorm + single-head spatial attention + proj + residual)
for Trainium2, SPMD across 8 NeuronCores.

Sharding: data-parallel over batch (4 images) x 2-way split of query
positions per image => 8 cores.  Attention is computed per-image with the
full key/value set on every core, so there are no collectives.

Per-core algorithm (image b, query half h):
  - Spatial positions of the local image copy are rolled so the core's
    2048 query positions are always local positions [0, 2048); the host
    un-rolls when assembling the output (attention and GroupNorm are
    permutation-invariant over spatial positions).
  - x is loaded in fp8(e4m3) (1 MiB/core: the DMA queues move ~70 GB/s
    each, so input bytes directly gate startup).  GroupNorm stats are
    computed from the fp8 x, split between DVE bn_stats and ACT
    sum/sum-of-squares (activation accum_out) so the two engines halve
    the serial stats time.  GroupNorm is folded into the projections:
    h = a*x + b with a folded into fp8 weights, b into projection biases.
  - wproj is folded into the v projection on the host; the remaining
    per-channel output constant (w2@b + wproj@bv + bproj) ships back as
    the tiny `b2o` output and is added on the host with the residual x.
  - k's projection bias is dropped (softmax-invariant).
  - All projections run in fp8: q/k as DoubleRow matmuls (contract all
    256 channels per pass), v as regular fp8 matmuls.
  - Attention in fp8 DoubleRow: one matmul per 128-key score tile.  exp
    uses a constant shift (softmax-invariant) to stay in fp8 range and is
    split across engines: most j-tiles on ACT (exact exp), a subset on
    DVE via a Schraudolph bit trick - q is pre-scaled by 8*log2e*SCALE so
    the e4m3 BITS of e^s are just round(sp + BITS_BIAS), one saturating
    tensor_scalar into a u8 view (~3% max err, on par with fp8 rounding).
  - PV with v^T stationary fp8 pairs accumulating aT[ch, i-block]; an
    all-ones fp8 stationary yields the softmax denominator broadcast
    across partitions for free; normalization via reciprocal_approx_fast.
  - Output is stored bf16 (halves the writeback; ~0.2% of the residual-
    free part) and upconverted on the host while adding x and b2o.
  - PE warm-up matmuls fill the initial DMA/stats wait so the HAM clock
    gate is released before the real matmul stream starts.
"""

import numpy as np
import ml_dtypes

import concourse.bacc as bacc
import concourse.bass as bass
import concourse.mybir as mybir
import concourse.tile as tile
from concourse.bass_utils import run_bass_kernel_spmd

F32 = mybir.dt.float32
BF16 = mybir.dt.bfloat16
FP8 = mybir.dt.float8e4
U8 = mybir.dt.uint8
DR = mybir.MatmulPerfMode.DoubleRow

C = 256          # channels
HW = 4096        # spatial positions (64*64)
B = 4            # batch
NCORES = 8
IH = HW // 2     # query positions per core
P = 128          # partitions
NCC = C // P     # channel chunks (2)
IBLK = 512       # query i-block
NIB = IH // IBLK # 4 i-blocks per core
NJT = HW // P    # 32 key tiles
NPAIR = NJT // 2 # 16 key-tile pairs (DoubleRow)
EPS = 1e-6
SCALE = 1.0 / 16.0  # 1/sqrt(C)
SHIFT = 3.0         # exp(s*SCALE - SHIFT): keeps e^s in fp8 e4m3 range
LOG2E = 1.4426950408889634
KQ = 8.0 * LOG2E * SCALE         # folded into q so DVE-exp is one op
ACT_SCALE = 1.0 / (8.0 * LOG2E)  # ACT exp sees s*SCALE = sp*ACT_SCALE
# Schraudolph fp8-bit exp for the DVE share (delta=-0.065 by simulation)
BITS_BIAS = 56.0 - 8.0 * LOG2E * SHIFT + 8.0 * (-0.065)
TRICK_JT = frozenset((2, 7, 12, 17, 22, 27, 30))  # j-tiles exp'd on DVE

NPBF16 = ml_dtypes.bfloat16
NPFP8 = ml_dtypes.float8_e4m3fn

_PROGRAM = None  # cached (nc)
LAST_RESULTS = None  # BassKernelResults of the most recent run (for test harness)
TRACE = False


def _build_program():
    nc = bacc.Bacc()

    xr_d = nc.declare_dram_parameter("xr", [C, HW], FP8, isOutput=False)
    wq_d = nc.declare_dram_parameter("wqt", [C, C], BF16, isOutput=False)
    wk_d = nc.declare_dram_parameter("wkt", [C, C], BF16, isOutput=False)
    w2_d = nc.declare_dram_parameter("w2t", [C, C], BF16, isOutput=False)
    bq_d = nc.declare_dram_parameter("bq", [C], F32, isOutput=False)
    b2h_d = nc.declare_dram_parameter("b2h", [C], F32, isOutput=False)  # wproj@bv+bproj
    gns_d = nc.declare_dram_parameter("gns", [C], F32, isOutput=False)
    gnb_d = nc.declare_dram_parameter("gnb", [C], F32, isOutput=False)
    out_d = nc.declare_dram_parameter("out", [C, IH], BF16, isOutput=True)
    b2o_d = nc.declare_dram_parameter("b2o", [C], F32, isOutput=True)

    with tile.TileContext(nc) as tc:
        with (
            tc.tile_pool(name="wt", bufs=1) as wt,
            tc.tile_pool(name="xp", bufs=1) as xp,
            tc.tile_pool(name="qkv", bufs=1) as qkv,
            tc.tile_pool(name="scr", bufs=2) as scr,
        ):
            # ---------- constants ----------
            # memsets on the vector queue: the other queues issue DMAs and
            # the warm-up matmuls gate on warm_rhs
            warm_rhs = wt.tile([P, 128], F32, tag="warm_rhs", name="warm_rhs")
            nc.vector.memset(warm_rhs, 0.0)
            G = wt.tile([P, P], F32, tag="G", name="G")
            nc.vector.memset(G, 0.0)
            nc.vector.memset(G[0:64, 0:64], 1.0 / 64.0)
            nc.vector.memset(G[64:128, 64:128], 1.0 / 64.0)
            eps_t = wt.tile([P, 1], F32, tag="eps", name="eps")
            nc.vector.memset(eps_t, EPS)
            shift_t = wt.tile([P, 1], F32, tag="shift", name="shift")
            nc.vector.memset(shift_t, -SHIFT)
            ones_pair = wt.tile([P, 2, P], FP8, tag="ones8", name="ones8")
            nc.vector.memset(ones_pair, 1.0)

            # ---------- x loads first (startup critical path) ----------
            # one dma_start costs ~0.6us of issue time and each queue's DMA
            # engine moves ~70 GB/s, so spread x over three queues; the
            # scalar (ACT) queue stays free for the stats/conversion ops
            x8 = xp.tile([P, NCC, HW], FP8, tag="x8", name="x8")
            _eng = [nc.sync, nc.gpsimd, nc.scalar]
            # high columns first: the ACT stats ops need cols 2560:4096 in
            # full before they can start, while DVE bn_stats walks slices
            for i, w in enumerate((2, 3, 0, 1)):
                for cc in range(NCC):
                    _eng[(i * NCC + cc) % 3].dma_start(
                        out=x8[:, cc, w * 1024:(w + 1) * 1024],
                        in_=xr_d[cc * P:(cc + 1) * P, w * 1024:(w + 1) * 1024],
                    )

            # ---------- load weights / params ----------
            w_sb = {}
            for name, d in (("q", wq_d), ("k", wk_d), ("v", w2_d)):
                for cc in range(NCC):
                    t = wt.tile([P, C], BF16, tag=f"w{name}{cc}", name=f"w{name}{cc}")
                    nc.sync.dma_start(out=t, in_=d[cc * P:(cc + 1) * P, :])
                    w_sb[name, cc] = t
            par_sb = {}
            for name, d in (("bq", bq_d), ("gns", gns_d), ("gnb", gnb_d)):
                for cc in range(NCC):
                    t = wt.tile([P, 1], F32, tag=f"{name}{cc}", name=f"{name}{cc}")
                    nc.gpsimd.dma_start(out=t, in_=d[cc * P:(cc + 1) * P].unsqueeze(1))
                    par_sb[name, cc] = t
            b2h_sb = wt.tile([1, C], F32, tag="b2h", name="b2h")
            nc.gpsimd.dma_start(out=b2h_sb, in_=b2h_d[:].unsqueeze(0))

            # ---------- GroupNorm stats (on fp8 x; var bias ~0.1%) ----------
            # split across DVE (bn_stats, cols 0:2560) and ACT (sum/sum-sq
            # via activation accum_out, cols 2560:4096)
            with tc.tile_pool(name="psA", bufs=2, space="PSUM") as psA:
                # PE warm-up while x DMA + stats run: fills idle time and
                # brings HAM out of the cold 1.2 GHz state before real work
                warm_ps = psA.tile([P, 128], F32, tag="warm", name="warm")
                for _ in range(26):
                    nc.tensor.matmul(warm_ps, G, warm_rhs, start=True, stop=True)
                a_sb, b_sb = [], []
                NSL = 5  # 512-col slices handled by DVE bn_stats
                st6s = [scr.tile([P, NSL, 6], F32, tag=f"st6{cc}", name=f"st6{cc}")
                        for cc in range(NCC)]
                for w in (4, 0, 1, 2, 3):  # slice 4 lands first (chunk 2)
                    for cc in range(NCC):
                        nc.vector.bn_stats(out=st6s[cc][:, w, :], in_=x8[:, cc, w * 512:(w + 1) * 512])
                accs, accq = {}, {}
                for cc in range(NCC):
                    sco = scr.tile([P, HW - NSL * 512], F32, tag="sco", name="sco")
                    accs[cc] = scr.tile([P, 1], F32, tag=f"accs{cc}", name=f"accs{cc}")
                    accq[cc] = scr.tile([P, 1], F32, tag=f"accq{cc}", name=f"accq{cc}")
                    nc.scalar.activation(out=sco, in_=x8[:, cc, NSL * 512:],
                                         func=mybir.ActivationFunctionType.Copy,
                                         accum_out=accs[cc])
                    nc.scalar.activation(out=sco, in_=x8[:, cc, NSL * 512:],
                                         func=mybir.ActivationFunctionType.Square,
                                         accum_out=accq[cc])
                FA = NSL * 512 / HW  # fraction covered by bn_stats
                for cc in range(NCC):
                    st6 = st6s[cc]
                    mv = scr.tile([P, 2], F32, tag="mv", name="mv")
                    nc.vector.bn_aggr(out=mv, in_=st6)
                    # combine DVE partial (mean_a, var_a over FA) with ACT
                    # partial sums: st3 = [mean, var, mean^2] per channel
                    st3 = scr.tile([P, 3], F32, tag="st3", name="st3")
                    t_s = scr.tile([P, 1], F32, tag="t_s", name="t_s")
                    nc.vector.tensor_scalar_mul(t_s, accs[cc], 1.0 / HW)
                    nc.vector.scalar_tensor_tensor(
                        out=st3[:, 0:1], in0=mv[:, 0:1], scalar=FA, in1=t_s,
                        op0=mybir.AluOpType.mult, op1=mybir.AluOpType.add)
                    t_q = scr.tile([P, 1], F32, tag="t_q", name="t_q")
                    nc.vector.tensor_scalar_mul(t_q, accq[cc], 1.0 / HW)
                    m2a = scr.tile([P, 1], F32, tag="m2a", name="m2a")
                    nc.vector.tensor_mul(m2a, mv[:, 0:1], mv[:, 0:1])
                    ex2a = scr.tile([P, 1], F32, tag="ex2a", name="ex2a")
                    nc.vector.tensor_add(ex2a, mv[:, 1:2], m2a)
                    ex2 = scr.tile([P, 1], F32, tag="ex2", name="ex2")
                    nc.vector.scalar_tensor_tensor(
                        out=ex2, in0=ex2a, scalar=FA, in1=t_q,
                        op0=mybir.AluOpType.mult, op1=mybir.AluOpType.add)
                    nc.vector.tensor_mul(st3[:, 2:3], st3[:, 0:1], st3[:, 0:1])
                    nc.vector.tensor_sub(st3[:, 1:2], ex2, st3[:, 2:3])
                    gp = psA.tile([P, 3], F32, tag="gp", name="gp")
                    nc.tensor.matmul(gp, G, st3, start=True, stop=True)
                    # group stats, broadcast per channel: mean, E[var], E[mean^2]
                    gs = scr.tile([P, 3], F32, tag="gs", name="gs")
                    nc.vector.tensor_copy(gs, gp)
                    t1 = scr.tile([P, 1], F32, tag="t1", name="t1")
                    nc.vector.tensor_mul(t1, gs[:, 0:1], gs[:, 0:1])
                    vg = scr.tile([P, 1], F32, tag="vg", name="vg")
                    nc.vector.tensor_add(vg, gs[:, 1:2], gs[:, 2:3])
                    nc.vector.tensor_sub(vg, vg, t1)
                    sd = scr.tile([P, 1], F32, tag="sd", name="sd")
                    nc.scalar.activation(out=sd, in_=vg, func=mybir.ActivationFunctionType.Sqrt, bias=eps_t)
                    rstd = scr.tile([P, 1], F32, tag="rstd", name="rstd")
                    nc.vector.reciprocal(rstd, sd)
                    a_t = wt.tile([P, 1], F32, tag=f"a{cc}", name=f"a{cc}")
                    nc.vector.tensor_mul(a_t, rstd, par_sb["gns", cc])
                    t2 = scr.tile([P, 1], F32, tag="t2", name="t2")
                    nc.vector.tensor_mul(t2, gs[:, 0:1], a_t)
                    b_t = wt.tile([P, 1], BF16, tag=f"b{cc}", name=f"b{cc}")
                    nc.vector.tensor_sub(b_t, par_sb["gnb", cc], t2)
                    a_sb.append(a_t)
                    b_sb.append(b_t)

                for _ in range(14):
                    nc.tensor.matmul(warm_ps, G, warm_rhs, start=True, stop=True)

                # ---------- fold GroupNorm scale into fp8 weights ----------
                # packed [128(in-ch chunk ki), 2(in-ch chunk ko), C(out)] for
                # DoubleRow; q additionally folds KQ so scores arrive
                # pre-scaled for the fp8-bit exp
                aq_sb = []
                for cc in range(NCC):
                    t = wt.tile([P, 1], F32, tag=f"aq{cc}", name=f"aq{cc}")
                    nc.vector.tensor_scalar_mul(t, a_sb[cc], KQ)
                    aq_sb.append(t)
                wf = {}
                for name in ("q", "k", "v"):
                    t = wt.tile([P, 2, C], FP8, tag=f"wf{name}", name=f"wf{name}")
                    for cc in range(NCC):
                        nc.vector.tensor_scalar_mul(
                            t[:, cc, :], w_sb[name, cc],
                            aq_sb[cc] if name == "q" else a_sb[cc])
                    wf[name] = t

                # ---------- effective biases (q bias also scaled by KQ) ----------
                be = {}
                for cc in range(NCC):
                    bp = psA.tile([P, 1], F32, tag="bp", name="bp")
                    nc.tensor.matmul(bp, w_sb["q", 0][:, cc * P:(cc + 1) * P], b_sb[0], start=True, stop=False)
                    nc.tensor.matmul(bp, w_sb["q", 1][:, cc * P:(cc + 1) * P], b_sb[1], start=False, stop=True)
                    t0 = scr.tile([P, 1], F32, tag="beq_r", name="beq_r")
                    nc.vector.tensor_add(t0, bp, par_sb["bq", cc])
                    t = wt.tile([P, 1], F32, tag=f"beq{cc}", name=f"beq{cc}")
                    nc.vector.tensor_scalar_mul(t, t0, KQ)
                    be["q", cc] = t
                # per-channel output constant w2@b + (wproj@bv + bproj),
                # shipped to the host (softmax rows sum to 1, so it can be
                # added after attention)
                b2p = psA.tile([1, C], F32, tag="b2p", name="b2p")
                nc.tensor.matmul(b2p, b_sb[0], w_sb["v", 0], start=True, stop=False)
                nc.tensor.matmul(b2p, b_sb[1], w_sb["v", 1], start=False, stop=True)
                b2row = wt.tile([1, C], F32, tag="b2row", name="b2row")
                nc.vector.tensor_add(b2row, b2p, b2h_sb)
                nc.sync.dma_start(out=b2o_d[:].unsqueeze(0), in_=b2row)

            # ---------- projections (fp8 in, fp8 packed out) ----------
            # q packed [128, i-block, 2(ch chunk), 512] (block-contiguous);
            # k packed [128, jt, 2(ch), 128]; vT packed [128, jt-pair, 2, C]
            q_sb = qkv.tile([P, NIB, NCC, IBLK], FP8, tag="q8", name="q8")
            k_sb = qkv.tile([P, NJT, NCC, P], FP8, tag="k8", name="k8")
            vT_sb = qkv.tile([P, NPAIR, 2, C], FP8, tag="vT8", name="vT8")

            # separate PSUM pools per projection so k/v matmuls never queue
            # behind q's (slow, DVE-paced) bias-adds on a shared tag
            with (
                tc.tile_pool(name="psQ", bufs=2, space="PSUM") as psQ,
                tc.tile_pool(name="psK", bufs=3, space="PSUM") as psK,
                tc.tile_pool(name="psV", bufs=3, space="PSUM") as psV,
            ):
                def qproj(ib):
                    for cc in range(NCC):
                        pq = psQ.tile([P, IBLK], F32, tag="pq", name="pq")
                        sl = slice(ib * IBLK, (ib + 1) * IBLK)
                        nc.tensor.matmul(pq, wf["q"][:, :, cc * P:(cc + 1) * P],
                                         x8[:, :, sl], start=True, stop=True, perf_mode=DR)
                        nc.vector.tensor_scalar_add(q_sb[:, ib, cc, :], pq, be["q", cc])

                qproj(0)  # block-0 scores depend on this q-block only
                # k and v interleaved so neither PSUM->fp8 conversion engine
                # (ACT for k, DVE for v mostly) stalls the PE
                for jb in range(HW // IBLK):
                    if jb in (1, 3, 5):
                        qproj((jb + 1) // 2)
                    for cc in range(NCC):
                        pk = psK.tile([P, IBLK], F32, tag="pk", name="pk")
                        sl = slice(jb * IBLK, (jb + 1) * IBLK)
                        nc.tensor.matmul(pk, wf["k"][:, :, cc * P:(cc + 1) * P],
                                         x8[:, :, sl], start=True, stop=True, perf_mode=DR)
                        # k's bias only adds a j-constant to each softmax row
                        # (q_i . bke), so it is dropped; fp8 pack on ACT
                        nc.scalar.copy(k_sb[:, 4 * jb:4 * jb + 4, cc, :], pk)
                    for g in (2 * jb, 2 * jb + 1):
                        # two j-tiles of v share one PSUM tile so the fp8
                        # pack is a single 512-wide op
                        pv = psV.tile([P, 2 * C], F32, tag="pv", name="pv")
                        for ko in range(2):
                            sl = slice((2 * g + ko) * P, (2 * g + ko + 1) * P)
                            nc.tensor.matmul(pv[:, ko * C:(ko + 1) * C], x8[:, 0, sl], wf["v"][:, 0, :], start=True, stop=False)
                            nc.tensor.matmul(pv[:, ko * C:(ko + 1) * C], x8[:, 1, sl], wf["v"][:, 1, :], start=False, stop=True)
                        if g % 4 == 3:
                            nc.scalar.copy(vT_sb[:, g, :, :], pv)
                        else:
                            nc.vector.tensor_copy(vT_sb[:, g, :, :], pv)

            # ---------- attention (fp8 DoubleRow) ----------
            with (
                tc.tile_pool(name="psS", bufs=5, space="PSUM") as psS,
                tc.tile_pool(name="psAT", bufs=1, space="PSUM") as psAT,
                tc.tile_pool(name="psDN", bufs=1, space="PSUM") as psDN,
                tc.tile_pool(name="eP", bufs=3) as eP,
                tc.tile_pool(name="oP", bufs=3) as oP,
                tc.tile_pool(name="rP", bufs=2) as rP,
            ):
                for ib in range(NIB):
                    isl = slice(ib * IBLK, (ib + 1) * IBLK)
                    sps = {}
                    eTs = {}

                    def scores(jt):
                        # one DoubleRow matmul per 128-key tile: contracts
                        # all 256 channels in a single pass
                        sp = psS.tile([P, IBLK], F32, tag="sp", name="sp")
                        nc.tensor.matmul(sp, k_sb[:, jt, :, :], q_sb[:, ib, :, :],
                                         start=True, stop=True, perf_mode=DR)
                        sps[jt] = sp

                    aT0 = psAT.tile([P, IBLK], F32, tag="aT0", name="aT0")
                    aT1 = psAT.tile([P, IBLK], F32, tag="aT1", name="aT1")
                    dnb = psDN.tile([P, IBLK], F32, tag="dnb", name="dnb")
                    for j0 in range(4):
                        scores(j0)
                    for jt in range(NJT):
                        if jt + 4 < NJT:
                            scores(jt + 4)
                        g, ko = jt // 2, jt % 2
                        if ko == 0:
                            eTs[g] = eP.tile([P, 2, IBLK], FP8, tag="eT", name="eT")
                        # shift keeps e^s within fp8 e4m3 range
                        # (softmax-invariant; un-done by the normalization)
                        if jt in TRICK_JT:
                            # fp8-bit exp on DVE: sp is pre-scaled (KQ fold),
                            # so e4m3 bits of e^s are just sp + BITS_BIAS
                            # (saturating u8 convert rounds + clamps at 0)
                            nc.vector.tensor_scalar_add(
                                eTs[g][:, ko, :].bitcast(U8), sps.pop(jt), BITS_BIAS)
                        else:
                            nc.scalar.activation(out=eTs[g][:, ko, :], in_=sps.pop(jt),
                                                 func=mybir.ActivationFunctionType.Exp,
                                                 scale=ACT_SCALE, bias=shift_t)
                        if ko == 1:
                            eT = eTs.pop(g)
                            st = (g == 0)
                            sp_ = (g == NPAIR - 1)
                            nc.tensor.matmul(aT0, vT_sb[:, g, :, 0:P], eT, start=st, stop=sp_, perf_mode=DR)
                            nc.tensor.matmul(aT1, vT_sb[:, g, :, P:C], eT, start=st, stop=sp_, perf_mode=DR)
                            # all-ones stationary: denominator, broadcast to
                            # all 128 partitions for free
                            nc.tensor.matmul(dnb, ones_pair, eT, start=st, stop=sp_, perf_mode=DR)
                    rec = rP.tile([P, IBLK], F32, tag="rec", name="rec")
                    nc.vector.reciprocal_approx_fast(out=rec, in_=dnb)
                    for cc, aT in ((0, aT0), (1, aT1)):
                        ot = oP.tile([P, IBLK], BF16, tag="ot", name="ot")
                        nc.vector.tensor_mul(ot, aT, rec)
                        if ib == NIB - 1:
                            # final block: split the writeback across two
                            # queues so the drain tail is halved
                            h = IBLK // 2
                            nc.sync.dma_start(out=out_d[cc * P:(cc + 1) * P, ib * IBLK:ib * IBLK + h], in_=ot[:, 0:h])
                            nc.gpsimd.dma_start(out=out_d[cc * P:(cc + 1) * P, ib * IBLK + h:(ib + 1) * IBLK], in_=ot[:, h:])
                        else:
                            nc.sync.dma_start(out=out_d[cc * P:(cc + 1) * P, isl], in_=ot)

    nc.finalize()
    return nc


def _get_program():
    global _PROGRAM
    if _PROGRAM is None:
        _PROGRAM = _build_program()
    return _PROGRAM


def kernel(x, gn_scale, gn_bias, wq, bq, wk, bk, wv, bv, wproj, bproj):
    global LAST_RESULTS
    x = np.asarray(x, dtype=np.float32)
    gn_scale = np.asarray(gn_scale, dtype=np.float32)
    gn_bias = np.asarray(gn_bias, dtype=np.float32)
    wq_ = np.asarray(wq, dtype=np.float32)
    wk_ = np.asarray(wk, dtype=np.float32)
    wv_ = np.asarray(wv, dtype=np.float32)
    wp_ = np.asarray(wproj, dtype=np.float32)
    bq_ = np.asarray(bq, dtype=np.float32)
    bv_ = np.asarray(bv, dtype=np.float32)
    bp_ = np.asarray(bproj, dtype=np.float32)

    b, c, h, w = x.shape
    assert (b, c, h * w) == (B, C, HW), x.shape

    w2 = (wp_.astype(np.float64) @ wv_.astype(np.float64)).astype(np.float32)
    b2h = (wp_.astype(np.float64) @ bv_.astype(np.float64)).astype(np.float32) + bp_

    wqt = np.ascontiguousarray(wq_.T).astype(NPBF16)
    wkt = np.ascontiguousarray(wk_.T).astype(NPBF16)
    w2t = np.ascontiguousarray(w2.T).astype(NPBF16)

    xf = x.reshape(B, C, HW)
    in_maps = []
    for core in range(NCORES):
        bi, hi = core // 2, core % 2
        xi = np.roll(xf[bi], -IH * hi, axis=1)
        in_maps.append({
            "xr": np.clip(xi, -240.0, 240.0).astype(NPFP8),
            "wqt": wqt, "wkt": wkt, "w2t": w2t,
            "bq": bq_, "b2h": b2h,
            "gns": gn_scale, "gnb": gn_bias,
        })

    nc = _get_program()
    res = run_bass_kernel_spmd(nc, in_maps, list(range(NCORES)), trace=TRACE)
    LAST_RESULTS = res

    out = np.empty((B, C, HW), dtype=np.float32)
    for core in range(NCORES):
        bi, hi = core // 2, core % 2
        out[bi][:, hi * IH:(hi + 1) * IH] = (
            res.results[core]["out"].astype(np.float32)
            + res.results[core]["b2o"][:, None]
        )
    out += xf
    return out.reshape(B, C, h, w)
